# revision 51
# baseline (speedup 1.0000x reference)
"""Trainium2 Bass kernel for nn_AttentionBlock (Set-Transformer MAB block).

Reference computation (per batch b):
    Qp = Q @ Wq.T + bq ; Kp = K @ Wk.T + bk ; Vp = K @ Wv.T + bv   (4 heads of 64)
    A  = softmax(Qp Kp^T / 8)  ;  ctx = A Vp
    O  = LN0(Qp + ctx) ;  O = O + relu(O @ Wo.T + bo) ;  out = LN1(O)

Sharding: data-parallel over (batch, query-half) -> 8 independent shards,
one per NeuronCore, no collectives.  Each core sees its 1024 queries, the
full 2048 keys of its batch, and all weights, shipped feature-major bf16.

Design notes (vs the first working kernel; measured 142.6us -> ~126.8us):
  * ACT exp is the pacing resource: 64 x [128,1024] exps ~ 71.5us at
    ~1113ns each.  Everything else targets exp-stream density: minimal
    startup to the first exp, PE side-tasks rationed to one per key-tile
    iteration, and the LN/MLP tail packed after the last exp.
  * Activation tables stay on the DEFAULT greedy sets (exp_and_others for
    Exp/Identity/Copy, one switch to sqrt_and_others pulled in by a pinned
    dummy Sqrt right after the last exp).  Pinning the combined
    natural_log_exp set measurably slows EVERY ACTIVATE by ~200ns.
  * DMA: inputs shipped partition-major with 2-6KB contiguous rows.  The
    16 SDMA engines round-robin all in-flight transfers, so non-critical
    chunks are gated behind the critical ones (Wqkv, QT, KT chunk0) via
    tiny gpsimd stub writes that add a WAW dependency.  Each transfer
    also pays a ~2us completion-receipt tail, so critical transfers are
    issued in parallel, not serialized.
  * HAM clock gate: the PE boots at 1.2GHz and re-throttles after ~3.4us
    of idling.  Dummy warm-up matmuls run during the DMA wait, and
    warm-keeper matmuls (data-pinned so the Tile scheduler cannot hoist
    them into the exp stream) bridge the phase-B -> tail gap.
  * tail math: LayerNorm is invariant to per-token affine maps and relu
    is positively homogeneous, so with m/sd from bn_stats (g0=1, b0=0):
        z   = O + sd*relu(rs*(O@Wo^T - m*wsum + sd*bo)) = O + relu(p4)
        out = (z - m1) * rs1
    where the correction rides the Wo matmul as one K=2 matmul with
    stationary rows [m; sd] against [-wsum; bo] (wsum negated by using a
    -1 column in its colsum matmul).  Non-trivial gamma/beta fall back to
    a numpy path (never exercised by this problem's setup_inputs).
  * The tail runs in pairs of query tiles (copies/relu+residual batched
    to [128,512]), normalizes on ACT's free affine (bias=-m1*rs1), and is
    Vector-throughput-bound at ~2.2us/pair.
"""

from contextlib import ExitStack

import ml_dtypes
import numpy as np

import concourse.bass as bass
import concourse.tile as tile
from concourse import bacc, mybir
from concourse.bass_utils import run_bass_kernel_spmd
from concourse.masks import make_identity

FP = mybir.dt.float32
BF = mybir.dt.bfloat16
AF = mybir.ActivationFunctionType
OP = mybir.AluOpType

B = 4
SQ_FULL = 2048   # queries per batch
SK = 2048        # keys per batch
D = 256
H = 4
DH = D // H      # 64
NCORES = 8
QSPLIT = 2
SQ = SQ_FULL // QSPLIT    # queries per core
NQT = SQ // 128           # 8 query tiles
NKT = SK // 128           # 16 key tiles
NDT = D // 128            # 2 feature tiles
QN = SQ // 512            # 2 query column blocks
KC = SK // 512            # 4 key column chunks
LN_EPS = 1e-5
SCALE = 0.125             # 1 / sqrt(DH)

def _emit(nc):
    # DRAM parameters: everything partition-major with fat contiguous rows.
    QTd = nc.declare_dram_parameter("QT", [128, QN * NDT * 512], BF, isOutput=False)
    KTd = nc.declare_dram_parameter("KT", [128, KC * NDT * 512], BF, isOutput=False)
    Wd = nc.declare_dram_parameter("Wall", [128, 4 * NDT * D], BF, isOutput=False)
    Browd = nc.declare_dram_parameter("brows", [1, 3 * D], BF, isOutput=False)
    V1 = {
        n: nc.declare_dram_parameter(n, [D], FP, isOutput=False)
        for n in ("bq", "bk")
    }
    out = nc.declare_dram_parameter("out", [SQ, D], BF, isOutput=True)

    with tile.TileContext(nc) as tc, ExitStack() as ctx:
        singles = ctx.enter_context(tc.tile_pool(name="singles", bufs=1))
        big = ctx.enter_context(tc.tile_pool(name="big", bufs=1))
        ex = ctx.enter_context(tc.tile_pool(name="ex", bufs=4))
        ctp = ctx.enter_context(tc.tile_pool(name="ctp", bufs=2))
        outp = ctx.enter_context(tc.tile_pool(name="outp", bufs=8))
        ztp = ctx.enter_context(tc.tile_pool(name="ztp", bufs=4))

        # QpT/QT are split into separate tiles per (dvt, n) / n so readers
        # wait only on the exact producer (Tile dep granularity is coarse
        # within one tile) — the first exp must not wait the n1 projection.
        QpTt = [[big.tile([128, 512], BF, name=f"qpt{d}{n}", tag=f"qpt{d}{n}")
                 for n in range(QN)] for d in range(NDT)]
        KpT = big.tile([128, NDT, SK], BF)
        Vp = big.tile([128, NKT, H, DH + 1], BF)
        O = big.tile([128, NQT, D], BF)
        OT = big.tile([128, NDT, SQ], BF)
        recips = big.tile([128, NQT, H], FP)
        denf = big.tile([128, NQT, H], FP, tag="denf")
        KT = big.tile([128, KC, NDT, 512], BF)
        QTt = [big.tile([128, NDT, 512], BF, name=f"qtt{n}", tag=f"qtt{n}")
               for n in range(QN)]
        WT = big.tile([128, 4, NDT, D], BF)     # Wq | Wk | Wv | Wo
        # tail stats
        msd = big.tile([128, NQT, 2], FP, tag="msd")    # [m0, v0->sd0]
        mv1 = big.tile([128, NQT, 2], FP, tag="mv1")
        rs1 = big.tile([128, NQT], FP, tag="rs1")
        lnt = big.tile([128, NQT], FP, tag="lnt")
        st4 = big.tile([128, NQT, H, 6], FP, tag="st4")
        bgrow = big.tile([2, NQT, 128], BF, tag="bgrow")
        wb2 = singles.tile([2, D], BF, tag="wb2")
        negones = singles.tile([128, 1], BF, tag="negones")
        ones_row = singles.tile([1, 128], BF, tag="ones_row")
        warm = singles.tile([128, 512], BF, tag="warm")  # PE warm-up src

        ident = singles.tile([128, 128], FP)
        identB = singles.tile([128, 128], BF)
        epst = singles.tile([128, 1], FP)
        ones41 = singles.tile([128, 4, 1], FP)

        def ppart(name):  # [D] dram -> [128, NDT] sbuf (feature-on-partition)
            t = singles.tile([128, NDT], FP, tag=f"pp_{name}")
            nc.gpsimd.dma_start(out=t[:], in_=V1[name][:].rearrange("(t p) -> p t", p=128))
            return t

        # ========== phase A: loads + critical-path projections ==============
        with ExitStack() as pctx:
            mm_ps = pctx.enter_context(tc.tile_pool(name="mmps", bufs=4, space="PSUM"))
            wu_ps = pctx.enter_context(tc.tile_pool(name="wups", bufs=1, space="PSUM"))

            # PE warm-up first: dummy matmuls on a memset tile into a dead
            # PSUM bank.  The HAM clock-gate lifts after ~3.4us of sustained
            # PE activity; the gpsimd memset lands ~6.5us (before the DMA
            # issues), so the PE is at 2.4GHz by ~10us when the projections
            # start.  No data deps -> they only occupy the in-order PE queue.
            nc.gpsimd.memset(warm[:], 1.0)
            wu = wu_ps.tile([128, 512], FP, tag="wu")
            for _ in range(6):
                nc.tensor.matmul(wu[:], warm[:, 0:128], warm[:], start=True, stop=True)

            # Critical-first DMA order, contention-controlled: the 16 SDMA
            # engines round-robin among all in-flight transfers, so only the
            # critical ones (Wqkv, KT chunk0, QT) are issued up front; the
            # tiny bias-row load sits between KTc0 and KTc1 on the sync
            # queue, delaying the non-critical KT chunks ~1us each.
            brow = singles.tile([1, 3 * D], BF, tag="brow")  # bq | bv | bo rows
            nc.sync.dma_start(out=brow[:], in_=Browd[:, :])
            for _ in range(6):
                nc.tensor.matmul(
                    wu[:, 0:256], brow[0:1, 0:128], brow[0:1, 0:256],
                    start=True, stop=True)
            nc.scalar.dma_start(
                out=WT[:, 0:3, :, :],
                in_=Wd[:, 0:3 * NDT * D].rearrange("p (w s d) -> p w s d", w=3, s=NDT))
            nc.sync.dma_start(
                out=KT[:, 0, :, :],
                in_=KTd[:, 0:1024].rearrange("p (s q) -> p s q", s=NDT))
            nc.scalar.dma_start(
                out=QTt[0][:, :, :],
                in_=QTd[:, 0:1024].rearrange("p (s q) -> p s q", s=NDT))
            nc.scalar.dma_start(
                out=QTt[1][:, :, :],
                in_=QTd[:, 1024:2048].rearrange("p (s q) -> p s q", s=NDT))
            bq_p = singles.tile([128, NDT], FP, tag="pp_bq")
            bk_p = singles.tile([128, NDT], FP, tag="pp_bk")
            nc.gpsimd.tensor_copy(out=bq_p[0:1, 0:1], in_=brow[0:1, 0:1])
            nc.gpsimd.dma_start(
                out=bq_p[:], in_=V1["bq"][:].rearrange("(t p) -> p t", p=128))
            nc.gpsimd.tensor_copy(out=bk_p[0:1, 0:1], in_=brow[0:1, 0:1])
            nc.gpsimd.dma_start(
                out=bk_p[:], in_=V1["bk"][:].rearrange("(t p) -> p t", p=128))
            # (non-critical DMA issues are sequenced below, after the
            # constants, gated on QTn1 by gpsimd stubs so they don't steal
            # SDMA bandwidth from the critical transfers)

            # constants (emitted after the DMA issues so they don't delay them)
            nc.vector.memset(ident[:], 0.0)
            make_identity(nc, ident, nomemset=True)
            nc.vector.memset(identB[:], 0.0)
            make_identity(nc, identB, nomemset=True)
            nc.vector.memset(epst, LN_EPS)
            nc.vector.memset(ones41[:], 1.0)
            nc.vector.memset(negones[:], -1.0)
            nc.vector.memset(ones_row[:], 1.0)

            # token-major bias broadcasts via rank-1 matmuls (replaces
            # 384KB of stride-0 HBM broadcast reads with a 1.5KB row load)
            bq_b = singles.tile([128, D], FP, tag="bb_bq")
            bv_b = singles.tile([128, D], FP, tag="bb_bv")
            for row, dst in ((0, bq_b), (1, bv_b)):
                bps = mm_ps.tile([128, 512], FP, tag="mm")
                nc.tensor.matmul(
                    bps[:, :D], ones_row[:], brow[0:1, row * D:(row + 1) * D],
                    start=True, stop=True)
                nc.vector.tensor_copy(out=dst[:], in_=bps[:, :D])
            bv_v = bv_b[:, :].rearrange("p (h d) -> p h d", h=H)

            # Non-critical transfers are SEQUENCED behind the critical ones
            # via tiny stub writes: the 16 SDMA engines round-robin among all
            # in-flight transfers at packet granularity, so an early issue
            # would steal ~half the bandwidth from Wqkv/QT/KTc0.  Each stub
            # reads 1 element of a critical tile (-> waits its DMA) and
            # dirties the non-critical destination (-> its DMA waits, WAW).
            nc.gpsimd.tensor_copy(out=KT[:, 1, 0, 0:1], in_=QTt[1][:, 0, 1:2])
            nc.sync.dma_start(
                out=KT[:, 1, :, :],
                in_=KTd[:, 1024:2048].rearrange("p (s q) -> p s q", s=NDT))
            nc.gpsimd.tensor_copy(out=WT[:, 3, 0, 0:1], in_=QTt[1][:, 0, 1:2])
            nc.sync.dma_start(
                out=WT[:, 3, :, :],
                in_=Wd[:, 3 * NDT * D:].rearrange("p (s d) -> p s d", s=NDT))
            nc.gpsimd.tensor_copy(out=KT[:, 2, 0, 0:1], in_=QTt[1][:, 0, 1:2])
            nc.sync.dma_start(
                out=KT[:, 2:4, :, :],
                in_=KTd[:, 2048:4096].rearrange("p (c s q) -> p c s q", c=2, s=NDT))

            def proj_chunk(pool, dvt, n, on_act):
                # QpTt[dvt][n] = Wq[dvt-block] @ QT[n] + bq
                ps = pool.tile([128, 512], FP, tag=("mm" if pool is mm_ps else "fil"))
                for dqt in range(NDT):
                    nc.tensor.matmul(
                        ps[:],
                        WT[:, 0, dqt, dvt * 128:(dvt + 1) * 128],
                        QTt[n][:, dqt, :],
                        start=(dqt == 0), stop=(dqt == NDT - 1))
                if on_act:
                    nc.scalar.activation(
                        out=QpTt[dvt][n][:], in_=ps[:],
                        func=AF.Identity, bias=bq_p[:, dvt:dvt + 1], scale=1.0)
                else:
                    nc.vector.tensor_scalar_add(
                        out=QpTt[dvt][n][:], in0=ps[:],
                        scalar1=bq_p[:, dvt:dvt + 1])

            def kproj(pool, dvt, c, on_act):
                # KpT[:, dvt, c*512:(c+1)*512]
                ps = pool.tile([128, 512], FP, tag=("mm" if pool is mm_ps else "fil"))
                for dqt in range(NDT):
                    nc.tensor.matmul(
                        ps[:],
                        WT[:, 1, dqt, dvt * 128:(dvt + 1) * 128],
                        KT[:, c, dqt, :],
                        start=(dqt == 0), stop=(dqt == NDT - 1))
                if on_act:
                    nc.scalar.activation(
                        out=KpT[:, dvt, c * 512:(c + 1) * 512], in_=ps[:],
                        func=AF.Identity, bias=bk_p[:, dvt:dvt + 1], scale=1.0)
                else:
                    nc.vector.tensor_scalar_add(
                        out=KpT[:, dvt, c * 512:(c + 1) * 512], in0=ps[:],
                        scalar1=bk_p[:, dvt:dvt + 1])

            def vp_pair(kts, pool):  # V projection for a pair of key tiles
                for kt in kts:
                    ps = pool.tile([128, 512], FP, tag=("mm" if pool is mm_ps else "fil"))
                    for dqt in range(NDT):
                        nc.tensor.matmul(
                            ps[:, :D],
                            KT[:, kt // 4, dqt, (kt % 4) * 128:(kt % 4 + 1) * 128],
                            WT[:, 2, dqt, :],
                            start=(dqt == 0), stop=(dqt == NDT - 1))
                    nc.vector.tensor_copy(out=Vp[:, kt, :, DH:DH + 1], in_=ones41[:])
                    nc.vector.tensor_add(
                        out=Vp[:, kt, :, 0:DH],
                        in0=ps[:, :D].rearrange("p (h d) -> p h d", h=H),
                        in1=bv_v)

            def obase(qt, pool):  # residual base O = Qp token-major
                ps = pool.tile([128, 512], FP, tag=("mm" if pool is mm_ps else "fil"))
                for dqt in range(NDT):
                    nc.tensor.matmul(
                        ps[:, :D],
                        QTt[qt // 4][:, dqt, (qt % 4) * 128:(qt % 4 + 1) * 128],
                        WT[:, 0, dqt, :],
                        start=(dqt == 0), stop=(dqt == NDT - 1))
                nc.vector.tensor_add(out=O[:, qt, :], in0=ps[:, :D], in1=bq_b[:])

            # critical path to the first exp: KpT(dvt0 c0) first (KT chunk0
            # lands ~0.8us before QT n0), then QpT(dvt0 n0/n1).  The Kp bias
            # add goes to Vector so it runs in parallel with the Qp identity
            # on ACT.
            kproj(mm_ps, 0, 0, False)
            proj_chunk(mm_ps, 0, 0, True)
            proj_chunk(mm_ps, 0, 1, True)

        # ========== phase B: attention + fillers ============================
        with ExitStack() as pctx:
            sc_ps = pctx.enter_context(tc.tile_pool(name="scps", bufs=2, space="PSUM"))
            cx_ps = pctx.enter_context(tc.tile_pool(name="cxps", bufs=1, space="PSUM"))
            aux_ps = pctx.enter_context(tc.tile_pool(name="auxps", bufs=2, space="PSUM"))

            # remaining projections, drip-fed into PE slack in dependency
            # order.  obase fillers MUST be emitted before head 0's merges
            # (the merges read+write O).  Entries later in the list may
            # depend on later DMA chunks.
            # Emission order = program order: a filler pumped at iteration i
            # is emitted before ctx(kt=i) and before mm_s(kt=i+2), so
            # vp_pair((2k,2k+1)) must be pumped at iteration <= 2k-1 and
            # kproj(0,c) at iteration <= 4c-2.
            fillers = []
            fillers.append(lambda: obase(0, aux_ps))                 # h0 kt0
            fillers.append(lambda: kproj(aux_ps, 0, 1, False))       # kt1
            fillers.append(lambda: vp_pair((4, 5), aux_ps))          # kt2
            fillers.append(lambda: vp_pair((6, 7), aux_ps))          # kt3
            fillers.append(lambda: kproj(aux_ps, 0, 2, False))       # kt4
            fillers.append(lambda: obase(1, aux_ps))                 # kt5
            fillers.append(lambda: vp_pair((8, 9), aux_ps))          # kt6
            fillers.append(lambda: vp_pair((10, 11), aux_ps))        # kt7
            fillers.append(lambda: kproj(aux_ps, 0, 3, False))       # kt8
            fillers.append(lambda: obase(2, aux_ps))                 # kt9
            fillers.append(lambda: vp_pair((12, 13), aux_ps))        # kt10
            fillers.append(lambda: vp_pair((14, 15), aux_ps))        # kt11
            for qt in range(3, NQT):
                fillers.append(lambda qt=qt: obase(qt, aux_ps))      # kt12..h1 kt0
            # dvt1 projections (needed from h2) + wsum prep, during h1
            for c in range(KC):
                fillers.append(lambda c=c: kproj(aux_ps, 1, c, False))
            for n in range(QN):
                fillers.append(lambda n=n: proj_chunk(aux_ps, 1, n, False))

            def wsum_prep():
                # wb2 row0 = -colsum(Wo^T) (negones lhsT), row1 = bo.
                # engines can't address a base partition of 1 -> wb2 row1
                # goes through a tiny SBUF->SBUF DMA.
                wsp = aux_ps.tile([1, 256], FP, tag="fil")
                for dvt in range(NDT):
                    nc.tensor.matmul(
                        wsp[:], negones[:], WT[:, 3, dvt, :],
                        start=(dvt == 0), stop=(dvt == NDT - 1))
                nc.vector.tensor_copy(out=wb2[0:1, :], in_=wsp[:])
                nc.gpsimd.dma_start(out=wb2[1:2, :], in_=brow[0:1, 2 * D:3 * D])

            fillers.append(wsum_prep)

            def pump(n):
                for _ in range(n):
                    if fillers:
                        fillers.pop(0)()

            def mm_s(h, kt, ns=None, sps=None):
                po = (h % 2) * DH
                dvt = h // 2
                if sps is None:
                    sps = sc_ps.tile([128, SQ], FP, tag="sc")
                for n in (range(SQ // 512) if ns is None else ns):
                    nc.tensor.matmul(
                        sps[:, n * 512:(n + 1) * 512],
                        KpT[po:po + DH, dvt, kt * 128:(kt + 1) * 128],
                        QpTt[dvt][n][po:po + DH, :],
                        start=True, stop=True)
                return sps

            def merge_qt(h, ctxTh, qt):
                # fold head h's ctx into O for one query tile + LN0 partials
                # (bf16 ctx -> 1-pass PE transpose; the denominator column is
                # re-staged fp32 for the bit-trick reciprocal)
                pmt = aux_ps.tile([128, DH + 1], BF, tag="fil")
                nc.tensor.transpose(
                    pmt[:], ctxTh[:, qt * 128:(qt + 1) * 128],
                    identB[:DH + 1, :DH + 1])
                nc.vector.tensor_copy(
                    out=denf[:, qt, h:h + 1], in_=pmt[:, DH:DH + 1])
                nc.vector.reciprocal_approx_fast(
                    out=recips[:, qt, h:h + 1], in_=denf[:, qt, h:h + 1])
                nc.vector.scalar_tensor_tensor(
                    out=O[:, qt, h * DH:(h + 1) * DH],
                    in0=pmt[:, 0:DH],
                    scalar=recips[:, qt, h:h + 1],
                    in1=O[:, qt, h * DH:(h + 1) * DH],
                    op0=OP.mult, op1=OP.add)
                nc.vector.bn_stats(
                    st4[:, qt, h, :], O[:, qt, h * DH:(h + 1) * DH])

            pre = None
            ctxTh_prev = None       # (h, ctxTh) whose merges are still queued
            for h in range(H - 1):
                cps = cx_ps.tile([DH + 1, SQ], FP, tag="cx")
                if pre is None:
                    # first exp split in two halves: the n0 half starts
                    # ~1us before QpT n1's scores are even computed
                    sps, nxt_pre = mm_s(h, 0, ns=(0,)), None
                    e0 = ex.tile([128, SQ], BF, tag="ex")
                    nc.scalar.activation(
                        out=e0[:, 0:512], in_=sps[:, 0:512], func=AF.Exp, scale=SCALE)
                    mm_s(h, 0, ns=(1,), sps=sps)
                    # V projections for the first key tiles must be emitted
                    # before ctx(kt0) reads Vp (in-order emission)
                    vp_pair((0, 1), aux_ps)
                    vp_pair((2, 3), aux_ps)
                else:
                    sps, nxt_pre = pre
                    e0 = None
                for kt in range(NKT):
                    if kt == 0 and nxt_pre is not None:
                        nxt = nxt_pre
                    else:
                        nxt = mm_s(h, kt + 1) if kt + 1 < NKT else None
                    if e0 is not None:
                        e = e0
                        nc.scalar.activation(
                            out=e[:, 512:SQ], in_=sps[:, 512:SQ],
                            func=AF.Exp, scale=SCALE)
                        e0 = None
                    else:
                        e = ex.tile([128, SQ], BF, tag="ex")
                        nc.scalar.activation(
                            out=e[:], in_=sps[:], func=AF.Exp, scale=SCALE)
                    # one PE-side side-task per iteration, BEFORE the ctx
                    # matmuls in the in-order PE queue: it runs in the bubble
                    # while ctx waits on this exp, instead of delaying
                    # scores(kt+2).  Merges of the previous head take the odd
                    # iterations, projection fillers the even ones — two
                    # tasks in one iteration overloads the PE beyond the exp
                    # pace (~1.35us/kt > 1.11us).  From h2 on the fillers are
                    # exhausted, so merges double up on early odd iterations,
                    # clearing the vector queue well before the tail starts.
                    if ctxTh_prev is not None and kt % 2 == 1:
                        if h >= 2:
                            if kt < 8:
                                merge_qt(ctxTh_prev[0], ctxTh_prev[1], kt - 1)
                                merge_qt(ctxTh_prev[0], ctxTh_prev[1], kt)
                        else:
                            merge_qt(ctxTh_prev[0], ctxTh_prev[1], kt // 2)
                    else:
                        pump(1)
                    for n in range(SQ // 512):
                        nc.tensor.matmul(
                            cps[:, n * 512:(n + 1) * 512],
                            Vp[:, kt, h, :],
                            e[:, n * 512:(n + 1) * 512],
                            start=(kt == 0), stop=(kt == NKT - 1))
                    if h == H - 1 and kt == NKT - 1:
                        sps_last, e_last = sps, e
                    sps = nxt

                # pre-emit the next head's first two score-tile matmuls so
                # they run during the merge/copy window (in-order PE queue).
                # h3 is processed per query half (below), so only its first
                # half's scores are pre-emitted.
                if h + 1 < H - 1:
                    pre = (mm_s(h + 1, 0), mm_s(h + 1, 1))
                else:
                    pre = (mm_s(3, 0, ns=(0,)), mm_s(3, 1, ns=(0,)))
                ctxTh = ctp.tile([DH + 1, SQ], BF, tag="ct")
                nc.vector.tensor_copy(out=ctxTh[:], in_=cps[:])
                ctxTh_prev = (h, ctxTh)

            # ---- h3: two query-half passes (32 half-iterations).  The
            # first half's ctx completes ~11us before the stream ends, so
            # its merges + LN0 stats (the Vector-heavy part of the tail)
            # run in the second half's side-task slots.  Costs 16 extra
            # exp-instruction overheads (~+2.4us ACT), frees ~2x that of
            # tail serialization.
            cps = cx_ps.tile([DH + 1, SQ], FP, tag="cx")
            ct3 = [ctp.tile([DH + 1, 256], BF, name=f"ct3_{i}",
                            tag=f"ct3_{i}") for i in range(4)]

            def merge_qt_tail(qt, pool):
                pmt = pool.tile([128, DH + 1], BF, tag=(
                    "fil" if pool is aux_ps else "mg"))
                nc.tensor.transpose(
                    pmt[:], ct3[qt // 2][:, (qt % 2) * 128:(qt % 2 + 1) * 128],
                    identB[:DH + 1, :DH + 1])
                nc.vector.tensor_copy(
                    out=denf[:, qt, 3:4], in_=pmt[:, DH:DH + 1])
                nc.vector.reciprocal_approx_fast(
                    out=recips[:, qt, 3:4], in_=denf[:, qt, 3:4])
                nc.vector.scalar_tensor_tensor(
                    out=O[:, qt, 3 * DH:4 * DH], in0=pmt[:, 0:DH],
                    scalar=recips[:, qt, 3:4], in1=O[:, qt, 3 * DH:4 * DH],
                    op0=OP.mult, op1=OP.add)
                nc.vector.bn_stats(st4[:, qt, 3, :], O[:, qt, 3 * DH:4 * DH])
                nc.vector.bn_aggr(msd[:, qt, :], st4[:, qt, :, :])

            sps, nxt = pre
            for i in range(2 * NKT):
                hf, kt = divmod(i, NKT)
                cur = sps
                sps = nxt
                nxt = (mm_s(3, (i + 2) % NKT, ns=((i + 2) // NKT,))
                       if i + 2 < 2 * NKT else None)
                e = ex.tile([128, SQ], BF, tag="ex")
                nc.scalar.activation(
                    out=e[:, 0:512], in_=cur[:, hf * 512:(hf + 1) * 512],
                    func=AF.Exp, scale=SCALE)
                if hf == 0:
                    # h2's merges, two per early odd slot
                    if i % 2 == 1 and i < 8:
                        merge_qt(ctxTh_prev[0], ctxTh_prev[1], i - 1)
                        merge_qt(ctxTh_prev[0], ctxTh_prev[1], i)
                else:
                    j = i - NKT
                    if j == 1:
                        # stage half-0's ctx (its accumulation is complete)
                        nc.vector.tensor_copy(out=ct3[0][:], in_=cps[:, 0:256])
                        nc.vector.tensor_copy(out=ct3[1][:], in_=cps[:, 256:512])
                    elif j in (3, 5, 7, 9):
                        merge_qt_tail((j - 3) // 2, aux_ps)
                nc.tensor.matmul(
                    cps[:, hf * 512:(hf + 1) * 512], Vp[:, kt, 3, :],
                    e[:, 0:512], start=(kt == 0), stop=(kt == NKT - 1))
                if i == 2 * NKT - 1:
                    sps_last, e_last = cur, e

            # pinned dummy Sqrt pulls the sqrt-table load in right after the
            # last exp, in parallel with the ctx staging on Vector
            sqscr = singles.tile([128, 1], FP, tag="sqscr")
            nc.scalar.activation(
                out=sqscr[:], in_=sps_last[:, 512:513],
                func=AF.Sqrt, bias=epst[:], scale=1.0)
            nc.vector.tensor_copy(out=ct3[2][:], in_=cps[:, 512:768])
            nc.vector.tensor_copy(out=ct3[3][:], in_=cps[:, 768:1024])

        # ========== phase C: h3 merges + LN0, MLP, LN1, store ===============
        # processed in pairs of query tiles: the elementwise/copy ops batch
        # to [128,512] (halving per-op overhead); stats stay per-qt.
        with ExitStack() as pctx:
            tr_ps = pctx.enter_context(tc.tile_pool(name="trps", bufs=2, space="PSUM"))
            wo_ps = pctx.enter_context(tc.tile_pool(name="wops", bufs=2, space="PSUM"))
            bg_ps = pctx.enter_context(tc.tile_pool(name="bgps", bufs=1, space="PSUM"))
            mg_ps = pctx.enter_context(tc.tile_pool(name="mgps", bufs=2, space="PSUM"))
            wk_ps = pctx.enter_context(tc.tile_pool(name="wkps", bufs=1, space="PSUM"))

            # PE warm-keepers: the HAM clock-gate re-throttles the PE to
            # 1.2GHz after a ~3.4us idle window, and the gap between the last
            # ctx matmul and the first tail transpose (waiting on the ctxTh
            # copies) is exactly such a window.  Dummy matmuls keep it warm;
            # they read the last e tile so the Tile scheduler cannot hoist
            # them into the exp stream (it moved no-dep dummies to ~90us).
            wk = wk_ps.tile([128, 256], FP, tag="wk")
            for _ in range(6):
                nc.tensor.matmul(
                    wk[:], warm[:, 0:128], e_last[:, 0:256], start=True, stop=True)

            def sd_group(qb, k):
                # msd[:, qb:qb+k, 1]: v0 -> sd0 = sqrt(v0 + eps), in place
                # (elementwise same-range in/out is stream-safe on ACT)
                nc.scalar.activation(
                    out=msd[:, qb:qb + k, 1], in_=msd[:, qb:qb + k, 1],
                    func=AF.Sqrt, bias=epst[:], scale=1.0)

            sd_group(0, 4)   # half-0's merges already ran in-stream
            for qp in range(NQT // 2):
                q0 = 2 * qp
                # O transposes (bf16, 1-pass) -> OT, one ACT copy per pair
                tr = tr_ps.tile([128, 2, 2, 128], BF, tag="tr")  # [dvt, j, q]
                for dvt in range(NDT):
                    for j in range(2):
                        nc.tensor.transpose(
                            tr[:, dvt, j, :],
                            O[:, q0 + j, dvt * 128:(dvt + 1) * 128], identB[:])
                nc.scalar.copy(
                    out=OT[:, :, q0 * 128:(q0 + 2) * 128], in_=tr[:, :, :, :])
                # rank-2 correction rows [m; sd] -> bgrow, one copy per pair
                bgp = bg_ps.tile([2, 2, 128], FP, tag="bg")
                for j in range(2):
                    nc.tensor.transpose(bgp[:, j, :], msd[:, q0 + j, :], ident[:])
                nc.scalar.copy(out=bgrow[:, q0:q0 + 2, :], in_=bgp[:, :, :])
                # Wo matmuls + corrections for both tiles of the pair
                wo = wo_ps.tile([128, 2, D], FP, tag="wo")
                for j in range(2):
                    for dvt in range(NDT):
                        nc.tensor.matmul(
                            wo[:, j, :], OT[:, dvt, (q0 + j) * 128:(q0 + j + 1) * 128],
                            WT[:, 3, dvt, :], start=(dvt == 0), stop=False)
                    nc.tensor.matmul(
                        wo[:, j, :], bgrow[:, q0 + j, :], wb2[:],
                        start=False, stop=True)
                # z = O + relu(p4), one fused vector op per pair
                zt = ztp.tile([128, 2, D], FP, tag="zt")
                nc.vector.scalar_tensor_tensor(
                    out=zt[:], in0=wo[:], scalar=0.0,
                    in1=O[:, q0:q0 + 2, :],
                    op0=OP.max, op1=OP.add)
                # pipeline: half-1's merges + sd before this pair's stats
                if q0 + 4 < NQT:
                    merge_qt_tail(q0 + 4, mg_ps)
                    merge_qt_tail(q0 + 5, mg_ps)
                    sd_group(q0 + 4, 2)
                # LN1 stats per qt (stats don't batch); rs1 sqrt batched per
                # pair; all normalizes ride ACT's free affine (bias=-m1*rs1)
                # to keep the Vector queue (the tail bottleneck) clear
                for j in range(2):
                    qt = q0 + j
                    st = ztp.tile([128, 6], FP, tag="st")
                    nc.vector.bn_stats(st[:], zt[:, j, :])
                    nc.vector.bn_aggr(mv1[:, qt, :], st[:])
                nc.scalar.activation(
                    out=lnt[:, q0:q0 + 2], in_=mv1[:, q0:q0 + 2, 1],
                    func=AF.Sqrt, bias=epst[:], scale=1.0)
                nc.vector.reciprocal_approx_fast(
                    out=rs1[:, q0:q0 + 2], in_=lnt[:, q0:q0 + 2])
                # mid-tail PE warm-keepers (HAM re-throttles on tail gaps)
                for _ in range(2):
                    nc.tensor.matmul(
                        wk[:], warm[:, 0:128],
                        OT[:, 0, q0 * 128:(q0 + 2) * 128],
                        start=True, stop=True)
                for j in range(2):
                    qt = q0 + j
                    f = outp.tile([128, D], BF, tag="f")
                    s1 = ztp.tile([128, 1], FP, tag="s1")
                    nc.vector.scalar_tensor_tensor(
                        out=s1[:], in0=mv1[:, qt, 0:1], scalar=-1.0,
                        in1=rs1[:, qt:qt + 1], op0=OP.mult, op1=OP.mult)
                    nc.scalar.activation(
                        out=f[:], in_=zt[:, j, :], func=AF.Identity,
                        scale=rs1[:, qt:qt + 1], bias=s1[:])
                    deng = (nc.sync, nc.scalar)[qt % 2]
                    deng.dma_start(out=out[qt * 128:(qt + 1) * 128, :], in_=f[:])

    return nc


_NC = {}


def build_nc():
    # NOTE: no act-table pinning — natural_log_exp_and_others measurably
    # slows every ACTIVATE by ~200ns (~+15us on the exp stream).  Default
    # greedy sets: Exp/Identity/Copy -> exp_and_others, Sqrt ->
    # sqrt_and_others with exactly one switch after the last exp.
    if "nc" not in _NC:
        nc = bacc.Bacc("TRN2", target_bir_lowering=False)
        _emit(nc)
        nc.compile()
        _NC["nc"] = nc
    return _NC["nc"]


def shard_inputs(Q, K, Wq, bq, Wk, bk, Wv, bv, Wo, bo, g0, beta0, g1, beta1):
    # host-side zero-FLOP layout transforms: ship everything feature-major bf16
    bf = ml_dtypes.bfloat16

    def wshape(w):  # [D, D] -> partition-major [128, NDT*D] (contiguous rows)
        wt = np.asarray(w).T.astype(bf)           # [ (s p), d ]
        return np.ascontiguousarray(
            wt.reshape(NDT, 128, D).transpose(1, 0, 2).reshape(128, NDT * D))

    def xshape(x, nblk):  # [S, D] -> [128, nblk, NDT, 512] -> [128, nblk*NDT*512]
        xt = np.asarray(x).T.astype(bf)           # [(s p), (n q)]
        return np.ascontiguousarray(
            xt.reshape(NDT, 128, nblk, 512).transpose(1, 2, 0, 3).reshape(128, -1))

    shared = {
        "Wall": np.ascontiguousarray(np.concatenate(
            [wshape(Wq), wshape(Wk), wshape(Wv), wshape(Wo)], axis=1)),
        "brows": np.ascontiguousarray(np.concatenate(
            [np.asarray(v, dtype=np.float32) for v in (bq, bv, bo)]
        ).astype(bf).reshape(1, 3 * D)),
        "bq": np.ascontiguousarray(np.asarray(bq, dtype=np.float32)),
        "bk": np.ascontiguousarray(np.asarray(bk, dtype=np.float32)),
    }
    in_maps = []
    for c in range(NCORES):
        b, half = c // QSPLIT, c % QSPLIT
        m = dict(shared)
        m["QT"] = xshape(Q[b, half * SQ:(half + 1) * SQ, :], QN)
        m["KT"] = xshape(K[b], KC)
        in_maps.append(m)
    return in_maps


def _gb_trivial(g0, beta0, g1, beta1):
    return bool(
        np.all(np.asarray(g0) == 1) and np.all(np.asarray(beta0) == 0)
        and np.all(np.asarray(g1) == 1) and np.all(np.asarray(beta1) == 0))


def _kernel_numpy(Q, K, Wq, bq, Wk, bk, Wv, bv, Wo, bo, g0, beta0, g1, beta1):
    # general gamma/beta fallback (the device pipeline folds LN affines away,
    # which is only valid for g=1, beta=0 — the shapes this problem ships)
    def ln(x, g, b):
        m = x.mean(-1, keepdims=True)
        v = ((x - m) ** 2).mean(-1, keepdims=True)
        return (x - m) / np.sqrt(v + LN_EPS) * g + b

    Qf = np.asarray(Q, np.float32)
    Kf = np.asarray(K, np.float32)
    Qp = Qf @ np.asarray(Wq, np.float32).T + bq
    Kp = Kf @ np.asarray(Wk, np.float32).T + bk
    Vpp = Kf @ np.asarray(Wv, np.float32).T + bv
    Bn, Sq, _ = Qp.shape
    out = np.empty((Bn, Sq, D), np.float32)
    for b_ in range(Bn):
        for h in range(H):
            sl = slice(h * DH, (h + 1) * DH)
            s = Qp[b_][:, sl] @ Kp[b_][:, sl].T * SCALE
            s -= s.max(-1, keepdims=True)
            e = np.exp(s)
            a = e / e.sum(-1, keepdims=True)
            out[b_][:, sl] = Qp[b_][:, sl] + a @ Vpp[b_][:, sl]
    o = ln(out, g0, beta0)
    o = o + np.maximum(o @ np.asarray(Wo, np.float32).T + bo, 0.0)
    return ln(o, g1, beta1)


def kernel(**inputs):
    if not _gb_trivial(inputs["g0"], inputs["beta0"], inputs["g1"], inputs["beta1"]):
        return _kernel_numpy(**inputs)
    nc = build_nc()
    in_maps = shard_inputs(**inputs)
    res = run_bass_kernel_spmd(nc, in_maps, core_ids=list(range(NCORES)))
    out = np.empty((B, SQ_FULL, D), np.float32)
    for c in range(NCORES):
        b, half = c // QSPLIT, c % QSPLIT
        out[b, half * SQ:(half + 1) * SQ, :] = res.results[c]["out"]
    return out


# revision 53
# speedup vs baseline: 1.0079x; 1.0079x over previous
"""Trainium2 Bass kernel for nn_AttentionBlock (Set-Transformer MAB block).

Reference computation (per batch b):
    Qp = Q @ Wq.T + bq ; Kp = K @ Wk.T + bk ; Vp = K @ Wv.T + bv   (4 heads of 64)
    A  = softmax(Qp Kp^T / 8)  ;  ctx = A Vp
    O  = LN0(Qp + ctx) ;  O = O + relu(O @ Wo.T + bo) ;  out = LN1(O)

Sharding: data-parallel over (batch, query-half) -> 8 independent shards,
one per NeuronCore, no collectives.  Each core sees its 1024 queries, the
full 2048 keys of its batch, and all weights, shipped feature-major bf16.

Design notes (vs the first working kernel; measured 142.6us -> ~126.8us):
  * ACT exp is the pacing resource: 64 x [128,1024] exps ~ 71.5us at
    ~1113ns each.  Everything else targets exp-stream density: minimal
    startup to the first exp, PE side-tasks rationed to one per key-tile
    iteration, and the LN/MLP tail packed after the last exp.
  * Activation tables stay on the DEFAULT greedy sets (exp_and_others for
    Exp/Identity/Copy, one switch to sqrt_and_others pulled in by a pinned
    dummy Sqrt right after the last exp).  Pinning the combined
    natural_log_exp set measurably slows EVERY ACTIVATE by ~200ns.
  * DMA: inputs shipped partition-major with 2-6KB contiguous rows.  The
    16 SDMA engines round-robin all in-flight transfers, so non-critical
    chunks are gated behind the critical ones (Wqkv, QT, KT chunk0) via
    tiny gpsimd stub writes that add a WAW dependency.  Each transfer
    also pays a ~2us completion-receipt tail, so critical transfers are
    issued in parallel, not serialized.
  * HAM clock gate: the PE boots at 1.2GHz and re-throttles after ~3.4us
    of idling.  Dummy warm-up matmuls run during the DMA wait, and
    warm-keeper matmuls (data-pinned so the Tile scheduler cannot hoist
    them into the exp stream) bridge the phase-B -> tail gap.
  * tail math: LayerNorm is invariant to per-token affine maps and relu
    is positively homogeneous, so with m/sd from bn_stats (g0=1, b0=0):
        z   = O + sd*relu(rs*(O@Wo^T - m*wsum + sd*bo)) = O + relu(p4)
        out = (z - m1) * rs1
    where the correction rides the Wo matmul as one K=2 matmul with
    stationary rows [m; sd] against [-wsum; bo] (wsum negated by using a
    -1 column in its colsum matmul).  Non-trivial gamma/beta fall back to
    a numpy path (never exercised by this problem's setup_inputs).
  * The tail runs in pairs of query tiles (copies/relu+residual batched
    to [128,512]), normalizes on ACT's free affine (bias=-m1*rs1), and is
    Vector-throughput-bound at ~2.2us/pair.
"""

from contextlib import ExitStack

import ml_dtypes
import numpy as np

import concourse.bass as bass
import concourse.tile as tile
from concourse import bacc, mybir
from concourse.bass_utils import run_bass_kernel_spmd
from concourse.masks import make_identity

FP = mybir.dt.float32
BF = mybir.dt.bfloat16
AF = mybir.ActivationFunctionType
OP = mybir.AluOpType

B = 4
SQ_FULL = 2048   # queries per batch
SK = 2048        # keys per batch
D = 256
H = 4
DH = D // H      # 64
NCORES = 8
QSPLIT = 2
SQ = SQ_FULL // QSPLIT    # queries per core
NQT = SQ // 128           # 8 query tiles
NKT = SK // 128           # 16 key tiles
NDT = D // 128            # 2 feature tiles
QN = SQ // 512            # 2 query column blocks
KC = SK // 512            # 4 key column chunks
LN_EPS = 1e-5
SCALE = 0.125             # 1 / sqrt(DH)

def _emit(nc):
    # DRAM parameters: everything partition-major with fat contiguous rows.
    QTd = nc.declare_dram_parameter("QT", [128, QN * NDT * 512], BF, isOutput=False)
    KTd = nc.declare_dram_parameter("KT", [128, KC * NDT * 512], BF, isOutput=False)
    Wd = nc.declare_dram_parameter("Wall", [128, 4 * NDT * D], BF, isOutput=False)
    Browd = nc.declare_dram_parameter("brows", [1, 3 * D], BF, isOutput=False)
    V1 = {
        n: nc.declare_dram_parameter(n, [D], FP, isOutput=False)
        for n in ("bq", "bk")
    }
    out = nc.declare_dram_parameter("out", [SQ, D], BF, isOutput=True)

    with tile.TileContext(nc) as tc, ExitStack() as ctx:
        singles = ctx.enter_context(tc.tile_pool(name="singles", bufs=1))
        big = ctx.enter_context(tc.tile_pool(name="big", bufs=1))
        ex = ctx.enter_context(tc.tile_pool(name="ex", bufs=4))
        ctp = ctx.enter_context(tc.tile_pool(name="ctp", bufs=2))
        outp = ctx.enter_context(tc.tile_pool(name="outp", bufs=8))
        ztp = ctx.enter_context(tc.tile_pool(name="ztp", bufs=4))

        # QpT/QT are split into separate tiles per (dvt, n) / n so readers
        # wait only on the exact producer (Tile dep granularity is coarse
        # within one tile) — the first exp must not wait the n1 projection.
        QpTt = [[big.tile([128, 512], BF, name=f"qpt{d}{n}", tag=f"qpt{d}{n}")
                 for n in range(QN)] for d in range(NDT)]
        KpT = big.tile([128, NDT, SK], BF)
        Vp = big.tile([128, NKT, H, DH + 1], BF)
        O = big.tile([128, NQT, D], BF)
        OT = big.tile([128, NDT, SQ], BF)
        recips = big.tile([128, NQT, H], FP)
        denf = big.tile([128, NQT, H], FP, tag="denf")
        KT = big.tile([128, KC, NDT, 512], BF)
        QTt = [big.tile([128, NDT, 512], BF, name=f"qtt{n}", tag=f"qtt{n}")
               for n in range(QN)]
        WT = big.tile([128, 4, NDT, D], BF)     # Wq | Wk | Wv | Wo
        # tail stats
        msd = big.tile([128, NQT, 2], FP, tag="msd")    # [m0, v0->sd0]
        mv1 = big.tile([128, NQT, 2], FP, tag="mv1")
        rs1 = big.tile([128, NQT], FP, tag="rs1")
        lnt = big.tile([128, NQT], FP, tag="lnt")
        st4 = big.tile([128, NQT, H, 6], FP, tag="st4")
        bgrow = big.tile([2, NQT, 128], BF, tag="bgrow")
        wb2 = singles.tile([2, D], BF, tag="wb2")
        negones = singles.tile([128, 1], BF, tag="negones")
        ones_row = singles.tile([1, 128], BF, tag="ones_row")
        warm = singles.tile([128, 512], BF, tag="warm")  # PE warm-up src

        ident = singles.tile([128, 128], FP)
        identB = singles.tile([128, 128], BF)
        epst = singles.tile([128, 1], FP)
        ones41 = singles.tile([128, 4, 1], FP)

        def ppart(name):  # [D] dram -> [128, NDT] sbuf (feature-on-partition)
            t = singles.tile([128, NDT], FP, tag=f"pp_{name}")
            nc.gpsimd.dma_start(out=t[:], in_=V1[name][:].rearrange("(t p) -> p t", p=128))
            return t

        # ========== phase A: loads + critical-path projections ==============
        with ExitStack() as pctx:
            mm_ps = pctx.enter_context(tc.tile_pool(name="mmps", bufs=4, space="PSUM"))
            wu_ps = pctx.enter_context(tc.tile_pool(name="wups", bufs=1, space="PSUM"))

            # PE warm-up first: dummy matmuls on a memset tile into a dead
            # PSUM bank.  The HAM clock-gate lifts after ~3.4us of sustained
            # PE activity; the gpsimd memset lands ~6.5us (before the DMA
            # issues), so the PE is at 2.4GHz by ~10us when the projections
            # start.  No data deps -> they only occupy the in-order PE queue.
            nc.gpsimd.memset(warm[:], 1.0)
            wu = wu_ps.tile([128, 512], FP, tag="wu")
            for _ in range(6):
                nc.tensor.matmul(wu[:], warm[:, 0:128], warm[:], start=True, stop=True)

            # Critical-first DMA order, contention-controlled: the 16 SDMA
            # engines round-robin among all in-flight transfers, so only the
            # critical ones (Wqkv, KT chunk0, QT) are issued up front; the
            # tiny bias-row load sits between KTc0 and KTc1 on the sync
            # queue, delaying the non-critical KT chunks ~1us each.
            brow = singles.tile([1, 3 * D], BF, tag="brow")  # bq | bv | bo rows
            nc.sync.dma_start(out=brow[:], in_=Browd[:, :])
            nc.scalar.dma_start(
                out=WT[:, 0:3, :, :],
                in_=Wd[:, 0:3 * NDT * D].rearrange("p (w s d) -> p w s d", w=3, s=NDT))
            nc.sync.dma_start(
                out=KT[:, 0, :, :],
                in_=KTd[:, 0:1024].rearrange("p (s q) -> p s q", s=NDT))
            nc.scalar.dma_start(
                out=QTt[0][:, :, :],
                in_=QTd[:, 0:1024].rearrange("p (s q) -> p s q", s=NDT))
            nc.scalar.dma_start(
                out=QTt[1][:, :, :],
                in_=QTd[:, 1024:2048].rearrange("p (s q) -> p s q", s=NDT))
            bq_p = ppart("bq")
            bk_p = ppart("bk")
            # (non-critical DMA issues are sequenced below, after the
            # constants, gated on QTn1 by gpsimd stubs so they don't steal
            # SDMA bandwidth from the critical transfers)

            # constants (emitted after the DMA issues so they don't delay them)
            nc.vector.memset(ident[:], 0.0)
            make_identity(nc, ident, nomemset=True)
            nc.vector.memset(identB[:], 0.0)
            make_identity(nc, identB, nomemset=True)
            nc.vector.memset(epst, LN_EPS)
            nc.vector.memset(ones41[:], 1.0)
            nc.vector.memset(negones[:], -1.0)
            nc.vector.memset(ones_row[:], 1.0)

            # token-major bias broadcasts via rank-1 matmuls (replaces
            # 384KB of stride-0 HBM broadcast reads with a 1.5KB row load)
            bq_b = singles.tile([128, D], FP, tag="bb_bq")
            bv_b = singles.tile([128, D], FP, tag="bb_bv")
            for row, dst in ((0, bq_b), (1, bv_b)):
                bps = mm_ps.tile([128, 512], FP, tag="mm")
                nc.tensor.matmul(
                    bps[:, :D], ones_row[:], brow[0:1, row * D:(row + 1) * D],
                    start=True, stop=True)
                nc.vector.tensor_copy(out=dst[:], in_=bps[:, :D])
            bv_v = bv_b[:, :].rearrange("p (h d) -> p h d", h=H)

            # Non-critical transfers are SEQUENCED behind the critical ones
            # via tiny stub writes: the 16 SDMA engines round-robin among all
            # in-flight transfers at packet granularity, so an early issue
            # would steal ~half the bandwidth from Wqkv/QT/KTc0.  Each stub
            # reads 1 element of a critical tile (-> waits its DMA) and
            # dirties the non-critical destination (-> its DMA waits, WAW).
            nc.gpsimd.tensor_copy(out=KT[:, 1, 0, 0:1], in_=QTt[1][:, 0, 1:2])
            nc.sync.dma_start(
                out=KT[:, 1, :, :],
                in_=KTd[:, 1024:2048].rearrange("p (s q) -> p s q", s=NDT))
            nc.gpsimd.tensor_copy(out=WT[:, 3, 0, 0:1], in_=QTt[1][:, 0, 1:2])
            nc.sync.dma_start(
                out=WT[:, 3, :, :],
                in_=Wd[:, 3 * NDT * D:].rearrange("p (s d) -> p s d", s=NDT))
            nc.gpsimd.tensor_copy(out=KT[:, 2, 0, 0:1], in_=QTt[1][:, 0, 1:2])
            nc.sync.dma_start(
                out=KT[:, 2:4, :, :],
                in_=KTd[:, 2048:4096].rearrange("p (c s q) -> p c s q", c=2, s=NDT))

            def proj_chunk(pool, dvt, n, on_act):
                # QpTt[dvt][n] = Wq[dvt-block] @ QT[n] + bq
                ps = pool.tile([128, 512], FP, tag=("mm" if pool is mm_ps else "fil"))
                for dqt in range(NDT):
                    nc.tensor.matmul(
                        ps[:],
                        WT[:, 0, dqt, dvt * 128:(dvt + 1) * 128],
                        QTt[n][:, dqt, :],
                        start=(dqt == 0), stop=(dqt == NDT - 1))
                if on_act:
                    nc.scalar.activation(
                        out=QpTt[dvt][n][:], in_=ps[:],
                        func=AF.Identity, bias=bq_p[:, dvt:dvt + 1], scale=1.0)
                else:
                    nc.vector.tensor_scalar_add(
                        out=QpTt[dvt][n][:], in0=ps[:],
                        scalar1=bq_p[:, dvt:dvt + 1])

            def kproj(pool, dvt, c, on_act):
                # KpT[:, dvt, c*512:(c+1)*512]
                ps = pool.tile([128, 512], FP, tag=("mm" if pool is mm_ps else "fil"))
                for dqt in range(NDT):
                    nc.tensor.matmul(
                        ps[:],
                        WT[:, 1, dqt, dvt * 128:(dvt + 1) * 128],
                        KT[:, c, dqt, :],
                        start=(dqt == 0), stop=(dqt == NDT - 1))
                if on_act:
                    nc.scalar.activation(
                        out=KpT[:, dvt, c * 512:(c + 1) * 512], in_=ps[:],
                        func=AF.Identity, bias=bk_p[:, dvt:dvt + 1], scale=1.0)
                else:
                    nc.vector.tensor_scalar_add(
                        out=KpT[:, dvt, c * 512:(c + 1) * 512], in0=ps[:],
                        scalar1=bk_p[:, dvt:dvt + 1])

            def vp_pair(kts, pool):  # V projection for a pair of key tiles
                for kt in kts:
                    ps = pool.tile([128, 512], FP, tag=("mm" if pool is mm_ps else "fil"))
                    for dqt in range(NDT):
                        nc.tensor.matmul(
                            ps[:, :D],
                            KT[:, kt // 4, dqt, (kt % 4) * 128:(kt % 4 + 1) * 128],
                            WT[:, 2, dqt, :],
                            start=(dqt == 0), stop=(dqt == NDT - 1))
                    nc.vector.tensor_copy(out=Vp[:, kt, :, DH:DH + 1], in_=ones41[:])
                    nc.vector.tensor_add(
                        out=Vp[:, kt, :, 0:DH],
                        in0=ps[:, :D].rearrange("p (h d) -> p h d", h=H),
                        in1=bv_v)

            def obase(qt, pool):  # residual base O = Qp token-major
                ps = pool.tile([128, 512], FP, tag=("mm" if pool is mm_ps else "fil"))
                for dqt in range(NDT):
                    nc.tensor.matmul(
                        ps[:, :D],
                        QTt[qt // 4][:, dqt, (qt % 4) * 128:(qt % 4 + 1) * 128],
                        WT[:, 0, dqt, :],
                        start=(dqt == 0), stop=(dqt == NDT - 1))
                nc.vector.tensor_add(out=O[:, qt, :], in0=ps[:, :D], in1=bq_b[:])

            # critical path to the first exp: KpT(dvt0 c0) first (KT chunk0
            # lands ~0.8us before QT n0), then QpT(dvt0 n0/n1).  The Kp bias
            # add goes to Vector so it runs in parallel with the Qp identity
            # on ACT.
            kproj(mm_ps, 0, 0, False)
            proj_chunk(mm_ps, 0, 0, True)
            proj_chunk(mm_ps, 0, 1, True)

        # ========== phase B: attention + fillers ============================
        with ExitStack() as pctx:
            sc_ps = pctx.enter_context(tc.tile_pool(name="scps", bufs=2, space="PSUM"))
            cx_ps = pctx.enter_context(tc.tile_pool(name="cxps", bufs=1, space="PSUM"))
            aux_ps = pctx.enter_context(tc.tile_pool(name="auxps", bufs=2, space="PSUM"))

            # remaining projections, drip-fed into PE slack in dependency
            # order.  obase fillers MUST be emitted before head 0's merges
            # (the merges read+write O).  Entries later in the list may
            # depend on later DMA chunks.
            # Emission order = program order: a filler pumped at iteration i
            # is emitted before ctx(kt=i) and before mm_s(kt=i+2), so
            # vp_pair((2k,2k+1)) must be pumped at iteration <= 2k-1 and
            # kproj(0,c) at iteration <= 4c-2.
            fillers = []
            fillers.append(lambda: obase(0, aux_ps))                 # h0 kt0
            fillers.append(lambda: kproj(aux_ps, 0, 1, False))       # kt1
            fillers.append(lambda: vp_pair((4, 5), aux_ps))          # kt2
            fillers.append(lambda: vp_pair((6, 7), aux_ps))          # kt3
            fillers.append(lambda: kproj(aux_ps, 0, 2, False))       # kt4
            fillers.append(lambda: obase(1, aux_ps))                 # kt5
            fillers.append(lambda: vp_pair((8, 9), aux_ps))          # kt6
            fillers.append(lambda: vp_pair((10, 11), aux_ps))        # kt7
            fillers.append(lambda: kproj(aux_ps, 0, 3, False))       # kt8
            fillers.append(lambda: obase(2, aux_ps))                 # kt9
            fillers.append(lambda: vp_pair((12, 13), aux_ps))        # kt10
            fillers.append(lambda: vp_pair((14, 15), aux_ps))        # kt11
            for qt in range(3, NQT):
                fillers.append(lambda qt=qt: obase(qt, aux_ps))      # kt12..h1 kt0
            # dvt1 projections (needed from h2) + wsum prep, during h1
            for c in range(KC):
                fillers.append(lambda c=c: kproj(aux_ps, 1, c, False))
            for n in range(QN):
                fillers.append(lambda n=n: proj_chunk(aux_ps, 1, n, False))

            def wsum_prep():
                # wb2 row0 = -colsum(Wo^T) (negones lhsT), row1 = bo.
                # engines can't address a base partition of 1 -> wb2 row1
                # goes through a tiny SBUF->SBUF DMA.
                wsp = aux_ps.tile([1, 256], FP, tag="fil")
                for dvt in range(NDT):
                    nc.tensor.matmul(
                        wsp[:], negones[:], WT[:, 3, dvt, :],
                        start=(dvt == 0), stop=(dvt == NDT - 1))
                nc.vector.tensor_copy(out=wb2[0:1, :], in_=wsp[:])
                nc.gpsimd.dma_start(out=wb2[1:2, :], in_=brow[0:1, 2 * D:3 * D])

            fillers.append(wsum_prep)

            def pump(n):
                for _ in range(n):
                    if fillers:
                        fillers.pop(0)()

            def mm_s(h, kt, ns=None, sps=None):
                po = (h % 2) * DH
                dvt = h // 2
                if sps is None:
                    sps = sc_ps.tile([128, SQ], FP, tag="sc")
                for n in (range(SQ // 512) if ns is None else ns):
                    nc.tensor.matmul(
                        sps[:, n * 512:(n + 1) * 512],
                        KpT[po:po + DH, dvt, kt * 128:(kt + 1) * 128],
                        QpTt[dvt][n][po:po + DH, :],
                        start=True, stop=True)
                return sps

            def merge_qt(h, ctxTh, qt):
                # fold head h's ctx into O for one query tile + LN0 partials
                # (bf16 ctx -> 1-pass PE transpose; the denominator column is
                # re-staged fp32 for the bit-trick reciprocal)
                pmt = aux_ps.tile([128, DH + 1], BF, tag="fil")
                nc.tensor.transpose(
                    pmt[:], ctxTh[:, qt * 128:(qt + 1) * 128],
                    identB[:DH + 1, :DH + 1])
                nc.vector.tensor_copy(
                    out=denf[:, qt, h:h + 1], in_=pmt[:, DH:DH + 1])
                nc.vector.reciprocal_approx_fast(
                    out=recips[:, qt, h:h + 1], in_=denf[:, qt, h:h + 1])
                nc.vector.scalar_tensor_tensor(
                    out=O[:, qt, h * DH:(h + 1) * DH],
                    in0=pmt[:, 0:DH],
                    scalar=recips[:, qt, h:h + 1],
                    in1=O[:, qt, h * DH:(h + 1) * DH],
                    op0=OP.mult, op1=OP.add)
                nc.vector.bn_stats(
                    st4[:, qt, h, :], O[:, qt, h * DH:(h + 1) * DH])

            pre = None
            ctxTh_prev = None       # (h, ctxTh) whose merges are still queued
            for h in range(H - 1):
                cps = cx_ps.tile([DH + 1, SQ], FP, tag="cx")
                if pre is None:
                    # first exp split in two halves: the n0 half starts
                    # ~1us before QpT n1's scores are even computed
                    sps, nxt_pre = mm_s(h, 0, ns=(0,)), None
                    e0 = ex.tile([128, SQ], BF, tag="ex")
                    nc.scalar.activation(
                        out=e0[:, 0:512], in_=sps[:, 0:512], func=AF.Exp, scale=SCALE)
                    mm_s(h, 0, ns=(1,), sps=sps)
                    # V projections for the first key tiles must be emitted
                    # before ctx(kt0) reads Vp (in-order emission)
                    vp_pair((0, 1), aux_ps)
                    vp_pair((2, 3), aux_ps)
                else:
                    sps, nxt_pre = pre
                    e0 = None
                for kt in range(NKT):
                    if kt == 0 and nxt_pre is not None:
                        nxt = nxt_pre
                    else:
                        nxt = mm_s(h, kt + 1) if kt + 1 < NKT else None
                    if e0 is not None:
                        e = e0
                        nc.scalar.activation(
                            out=e[:, 512:SQ], in_=sps[:, 512:SQ],
                            func=AF.Exp, scale=SCALE)
                        e0 = None
                    else:
                        e = ex.tile([128, SQ], BF, tag="ex")
                        nc.scalar.activation(
                            out=e[:], in_=sps[:], func=AF.Exp, scale=SCALE)
                    # one PE-side side-task per iteration, BEFORE the ctx
                    # matmuls in the in-order PE queue: it runs in the bubble
                    # while ctx waits on this exp, instead of delaying
                    # scores(kt+2).  Merges of the previous head take the odd
                    # iterations, projection fillers the even ones — two
                    # tasks in one iteration overloads the PE beyond the exp
                    # pace (~1.35us/kt > 1.11us).  From h2 on the fillers are
                    # exhausted, so merges double up on early odd iterations,
                    # clearing the vector queue well before the tail starts.
                    if ctxTh_prev is not None and kt % 2 == 1:
                        if h >= 2:
                            if kt < 8:
                                merge_qt(ctxTh_prev[0], ctxTh_prev[1], kt - 1)
                                merge_qt(ctxTh_prev[0], ctxTh_prev[1], kt)
                        else:
                            merge_qt(ctxTh_prev[0], ctxTh_prev[1], kt // 2)
                    else:
                        pump(1)
                    for n in range(SQ // 512):
                        nc.tensor.matmul(
                            cps[:, n * 512:(n + 1) * 512],
                            Vp[:, kt, h, :],
                            e[:, n * 512:(n + 1) * 512],
                            start=(kt == 0), stop=(kt == NKT - 1))
                    if h == H - 1 and kt == NKT - 1:
                        sps_last, e_last = sps, e
                    sps = nxt

                # pre-emit the next head's first two score-tile matmuls so
                # they run during the merge/copy window (in-order PE queue).
                # h3 is processed per query half (below), so only its first
                # half's scores are pre-emitted.
                if h + 1 < H - 1:
                    pre = (mm_s(h + 1, 0), mm_s(h + 1, 1))
                else:
                    pre = (mm_s(3, 0, ns=(0,)), mm_s(3, 1, ns=(0,)))
                ctxTh = ctp.tile([DH + 1, SQ], BF, tag="ct")
                nc.vector.tensor_copy(out=ctxTh[:], in_=cps[:])
                ctxTh_prev = (h, ctxTh)

            # ---- h3: two query-half passes (32 half-iterations).  The
            # first half's ctx completes ~11us before the stream ends, so
            # its merges + LN0 stats (the Vector-heavy part of the tail)
            # run in the second half's side-task slots.  Costs 16 extra
            # exp-instruction overheads (~+2.4us ACT), frees ~2x that of
            # tail serialization.
            cps = cx_ps.tile([DH + 1, SQ], FP, tag="cx")
            ct3 = [ctp.tile([DH + 1, 256], BF, name=f"ct3_{i}",
                            tag=f"ct3_{i}") for i in range(4)]

            def merge_qt_tail(qt, pool):
                pmt = pool.tile([128, DH + 1], BF, tag=(
                    "fil" if pool is aux_ps else "mg"))
                nc.tensor.transpose(
                    pmt[:], ct3[qt // 2][:, (qt % 2) * 128:(qt % 2 + 1) * 128],
                    identB[:DH + 1, :DH + 1])
                nc.vector.tensor_copy(
                    out=denf[:, qt, 3:4], in_=pmt[:, DH:DH + 1])
                nc.vector.reciprocal_approx_fast(
                    out=recips[:, qt, 3:4], in_=denf[:, qt, 3:4])
                nc.vector.scalar_tensor_tensor(
                    out=O[:, qt, 3 * DH:4 * DH], in0=pmt[:, 0:DH],
                    scalar=recips[:, qt, 3:4], in1=O[:, qt, 3 * DH:4 * DH],
                    op0=OP.mult, op1=OP.add)
                nc.vector.bn_stats(st4[:, qt, 3, :], O[:, qt, 3 * DH:4 * DH])
                nc.vector.bn_aggr(msd[:, qt, :], st4[:, qt, :, :])

            sps, nxt = pre
            for i in range(2 * NKT):
                hf, kt = divmod(i, NKT)
                cur = sps
                sps = nxt
                nxt = (mm_s(3, (i + 2) % NKT, ns=((i + 2) // NKT,))
                       if i + 2 < 2 * NKT else None)
                e = ex.tile([128, SQ], BF, tag="ex")
                nc.scalar.activation(
                    out=e[:, 0:512], in_=cur[:, hf * 512:(hf + 1) * 512],
                    func=AF.Exp, scale=SCALE)
                if hf == 0:
                    # h2's merges, two per early odd slot
                    if i % 2 == 1 and i < 8:
                        merge_qt(ctxTh_prev[0], ctxTh_prev[1], i - 1)
                        merge_qt(ctxTh_prev[0], ctxTh_prev[1], i)
                else:
                    j = i - NKT
                    if j == 1:
                        # stage half-0's ctx (its accumulation is complete)
                        nc.vector.tensor_copy(out=ct3[0][:], in_=cps[:, 0:256])
                        nc.vector.tensor_copy(out=ct3[1][:], in_=cps[:, 256:512])
                    elif j in (3, 5, 7, 9):
                        merge_qt_tail((j - 3) // 2, aux_ps)
                nc.tensor.matmul(
                    cps[:, hf * 512:(hf + 1) * 512], Vp[:, kt, 3, :],
                    e[:, 0:512], start=(kt == 0), stop=(kt == NKT - 1))
                if i == 2 * NKT - 1:
                    sps_last, e_last = cur, e

            # pinned dummy Sqrt pulls the sqrt-table load in right after the
            # last exp, in parallel with the ctx staging on Vector
            sqscr = singles.tile([128, 1], FP, tag="sqscr")
            nc.scalar.activation(
                out=sqscr[:], in_=sps_last[:, 512:513],
                func=AF.Sqrt, bias=epst[:], scale=1.0)
            nc.vector.tensor_copy(out=ct3[2][:], in_=cps[:, 512:768])
            nc.vector.tensor_copy(out=ct3[3][:], in_=cps[:, 768:1024])

        # ========== phase C: h3 merges + LN0, MLP, LN1, store ===============
        # processed in pairs of query tiles: the elementwise/copy ops batch
        # to [128,512] (halving per-op overhead); stats stay per-qt.
        with ExitStack() as pctx:
            tr_ps = pctx.enter_context(tc.tile_pool(name="trps", bufs=2, space="PSUM"))
            wo_ps = pctx.enter_context(tc.tile_pool(name="wops", bufs=2, space="PSUM"))
            bg_ps = pctx.enter_context(tc.tile_pool(name="bgps", bufs=1, space="PSUM"))
            mg_ps = pctx.enter_context(tc.tile_pool(name="mgps", bufs=2, space="PSUM"))
            wk_ps = pctx.enter_context(tc.tile_pool(name="wkps", bufs=1, space="PSUM"))

            # PE warm-keepers: the HAM clock-gate re-throttles the PE to
            # 1.2GHz after a ~3.4us idle window, and the gap between the last
            # ctx matmul and the first tail transpose (waiting on the ctxTh
            # copies) is exactly such a window.  Dummy matmuls keep it warm;
            # they read the last e tile so the Tile scheduler cannot hoist
            # them into the exp stream (it moved no-dep dummies to ~90us).
            wk = wk_ps.tile([128, 256], FP, tag="wk")
            for _ in range(6):
                nc.tensor.matmul(
                    wk[:], warm[:, 0:128], e_last[:, 0:256], start=True, stop=True)

            def sd_group(qb, k):
                # msd[:, qb:qb+k, 1]: v0 -> sd0 = sqrt(v0 + eps), in place
                # (elementwise same-range in/out is stream-safe on ACT)
                nc.scalar.activation(
                    out=msd[:, qb:qb + k, 1], in_=msd[:, qb:qb + k, 1],
                    func=AF.Sqrt, bias=epst[:], scale=1.0)

            sd_group(0, 4)   # half-0's merges already ran in-stream
            for qp in range(NQT // 2):
                q0 = 2 * qp
                # O transposes (bf16, 1-pass) -> OT, one ACT copy per pair
                tr = tr_ps.tile([128, 2, 2, 128], BF, tag="tr")  # [dvt, j, q]
                for dvt in range(NDT):
                    for j in range(2):
                        nc.tensor.transpose(
                            tr[:, dvt, j, :],
                            O[:, q0 + j, dvt * 128:(dvt + 1) * 128], identB[:])
                nc.scalar.copy(
                    out=OT[:, :, q0 * 128:(q0 + 2) * 128], in_=tr[:, :, :, :])
                # rank-2 correction rows [m; sd] -> bgrow, one copy per pair
                bgp = bg_ps.tile([2, 2, 128], FP, tag="bg")
                for j in range(2):
                    nc.tensor.transpose(bgp[:, j, :], msd[:, q0 + j, :], ident[:])
                nc.scalar.copy(out=bgrow[:, q0:q0 + 2, :], in_=bgp[:, :, :])
                # Wo matmuls + corrections for both tiles of the pair
                wo = wo_ps.tile([128, 2, D], FP, tag="wo")
                for j in range(2):
                    for dvt in range(NDT):
                        nc.tensor.matmul(
                            wo[:, j, :], OT[:, dvt, (q0 + j) * 128:(q0 + j + 1) * 128],
                            WT[:, 3, dvt, :], start=(dvt == 0), stop=False)
                    nc.tensor.matmul(
                        wo[:, j, :], bgrow[:, q0 + j, :], wb2[:],
                        start=False, stop=True)
                # z = O + relu(p4), one fused vector op per pair
                zt = ztp.tile([128, 2, D], FP, tag="zt")
                nc.vector.scalar_tensor_tensor(
                    out=zt[:], in0=wo[:], scalar=0.0,
                    in1=O[:, q0:q0 + 2, :],
                    op0=OP.max, op1=OP.add)
                # pipeline: half-1's merges + sd before this pair's stats
                if q0 + 4 < NQT:
                    merge_qt_tail(q0 + 4, mg_ps)
                    merge_qt_tail(q0 + 5, mg_ps)
                    sd_group(q0 + 4, 2)
                # LN1 stats per qt (stats don't batch); rs1 sqrt batched per
                # pair; all normalizes ride ACT's free affine (bias=-m1*rs1)
                # to keep the Vector queue (the tail bottleneck) clear
                for j in range(2):
                    qt = q0 + j
                    st = ztp.tile([128, 6], FP, tag="st")
                    nc.vector.bn_stats(st[:], zt[:, j, :])
                    nc.vector.bn_aggr(mv1[:, qt, :], st[:])
                nc.scalar.activation(
                    out=lnt[:, q0:q0 + 2], in_=mv1[:, q0:q0 + 2, 1],
                    func=AF.Sqrt, bias=epst[:], scale=1.0)
                nc.vector.reciprocal_approx_fast(
                    out=rs1[:, q0:q0 + 2], in_=lnt[:, q0:q0 + 2])
                # mid-tail PE warm-keepers (HAM re-throttles on tail gaps)
                for _ in range(2):
                    nc.tensor.matmul(
                        wk[:], warm[:, 0:128],
                        OT[:, 0, q0 * 128:(q0 + 2) * 128],
                        start=True, stop=True)
                for j in range(2):
                    qt = q0 + j
                    f = outp.tile([128, D], BF, tag="f")
                    s1 = ztp.tile([128, 1], FP, tag="s1")
                    nc.vector.scalar_tensor_tensor(
                        out=s1[:], in0=mv1[:, qt, 0:1], scalar=-1.0,
                        in1=rs1[:, qt:qt + 1], op0=OP.mult, op1=OP.mult)
                    nc.scalar.activation(
                        out=f[:], in_=zt[:, j, :], func=AF.Identity,
                        scale=rs1[:, qt:qt + 1], bias=s1[:])
                    # all stores on the sync queue: a DMA_DIRECT2D issue
                    # occupies its queue ~600ns, and the scalar queue is the
                    # ACT engine — the tail's bottleneck (OT/bgrow copies,
                    # sqrts, normalizes).  Sync is idle here.
                    nc.sync.dma_start(
                        out=out[qt * 128:(qt + 1) * 128, :], in_=f[:])

    return nc


_NC = {}


def build_nc():
    # NOTE: no act-table pinning — natural_log_exp_and_others measurably
    # slows every ACTIVATE by ~200ns (~+15us on the exp stream).  Default
    # greedy sets: Exp/Identity/Copy -> exp_and_others, Sqrt ->
    # sqrt_and_others with exactly one switch after the last exp.
    if "nc" not in _NC:
        nc = bacc.Bacc("TRN2", target_bir_lowering=False)
        _emit(nc)
        nc.compile()
        _NC["nc"] = nc
    return _NC["nc"]


def shard_inputs(Q, K, Wq, bq, Wk, bk, Wv, bv, Wo, bo, g0, beta0, g1, beta1):
    # host-side zero-FLOP layout transforms: ship everything feature-major bf16
    bf = ml_dtypes.bfloat16

    def wshape(w):  # [D, D] -> partition-major [128, NDT*D] (contiguous rows)
        wt = np.asarray(w).T.astype(bf)           # [ (s p), d ]
        return np.ascontiguousarray(
            wt.reshape(NDT, 128, D).transpose(1, 0, 2).reshape(128, NDT * D))

    def xshape(x, nblk):  # [S, D] -> [128, nblk, NDT, 512] -> [128, nblk*NDT*512]
        xt = np.asarray(x).T.astype(bf)           # [(s p), (n q)]
        return np.ascontiguousarray(
            xt.reshape(NDT, 128, nblk, 512).transpose(1, 2, 0, 3).reshape(128, -1))

    shared = {
        "Wall": np.ascontiguousarray(np.concatenate(
            [wshape(Wq), wshape(Wk), wshape(Wv), wshape(Wo)], axis=1)),
        "brows": np.ascontiguousarray(np.concatenate(
            [np.asarray(v, dtype=np.float32) for v in (bq, bv, bo)]
        ).astype(bf).reshape(1, 3 * D)),
        "bq": np.ascontiguousarray(np.asarray(bq, dtype=np.float32)),
        "bk": np.ascontiguousarray(np.asarray(bk, dtype=np.float32)),
    }
    in_maps = []
    for c in range(NCORES):
        b, half = c // QSPLIT, c % QSPLIT
        m = dict(shared)
        m["QT"] = xshape(Q[b, half * SQ:(half + 1) * SQ, :], QN)
        m["KT"] = xshape(K[b], KC)
        in_maps.append(m)
    return in_maps


def _gb_trivial(g0, beta0, g1, beta1):
    return bool(
        np.all(np.asarray(g0) == 1) and np.all(np.asarray(beta0) == 0)
        and np.all(np.asarray(g1) == 1) and np.all(np.asarray(beta1) == 0))


def _kernel_numpy(Q, K, Wq, bq, Wk, bk, Wv, bv, Wo, bo, g0, beta0, g1, beta1):
    # general gamma/beta fallback (the device pipeline folds LN affines away,
    # which is only valid for g=1, beta=0 — the shapes this problem ships)
    def ln(x, g, b):
        m = x.mean(-1, keepdims=True)
        v = ((x - m) ** 2).mean(-1, keepdims=True)
        return (x - m) / np.sqrt(v + LN_EPS) * g + b

    Qf = np.asarray(Q, np.float32)
    Kf = np.asarray(K, np.float32)
    Qp = Qf @ np.asarray(Wq, np.float32).T + bq
    Kp = Kf @ np.asarray(Wk, np.float32).T + bk
    Vpp = Kf @ np.asarray(Wv, np.float32).T + bv
    Bn, Sq, _ = Qp.shape
    out = np.empty((Bn, Sq, D), np.float32)
    for b_ in range(Bn):
        for h in range(H):
            sl = slice(h * DH, (h + 1) * DH)
            s = Qp[b_][:, sl] @ Kp[b_][:, sl].T * SCALE
            s -= s.max(-1, keepdims=True)
            e = np.exp(s)
            a = e / e.sum(-1, keepdims=True)
            out[b_][:, sl] = Qp[b_][:, sl] + a @ Vpp[b_][:, sl]
    o = ln(out, g0, beta0)
    o = o + np.maximum(o @ np.asarray(Wo, np.float32).T + bo, 0.0)
    return ln(o, g1, beta1)


def kernel(**inputs):
    if not _gb_trivial(inputs["g0"], inputs["beta0"], inputs["g1"], inputs["beta1"]):
        return _kernel_numpy(**inputs)
    nc = build_nc()
    in_maps = shard_inputs(**inputs)
    res = run_bass_kernel_spmd(nc, in_maps, core_ids=list(range(NCORES)))
    out = np.empty((B, SQ_FULL, D), np.float32)
    for c in range(NCORES):
        b, half = c // QSPLIT, c % QSPLIT
        out[b, half * SQ:(half + 1) * SQ, :] = res.results[c]["out"]
    return out
